# revision 34
# baseline (speedup 1.0000x reference)
"""Trainium2 Bass kernel for nn_Conv2d_20590073217670.

Conv2d: input [32,64,64,64] (NCHW), weight [576,128] (unfold layout:
row = ci*9 + a*3 + b for tap (a,b)), bias [1,128,1,1], stride 1, pad 1.
Output [32,128,64,64].

Strategy: data-parallel over batch — 4 images per NeuronCore, 8 cores.
The host pre-casts the input to bf16 and pre-pads it into
xp[n, c, t, j] = img[n, c, t-1, j-1] (zero border rows/cols), so the
device builds its two padded [128, 66, 66] GEMM layouts without any
cast work:
  xb: parts 0:64  <- DMA xp[n]           (= img[c, r-1, j-1])
      parts 64:128 <- DMA xp[n][:,1:65]  (= img[c, r,   j-1], one row up)
  xc: parts 0:64  <- DVE copy of xb lower
      parts 64:128 <- Act copy of xb lower shifted one col left
                                         (= img[c, r-1, j  ])
Per 8-row output block, 5 matmul passes, each a uniform full [8, 64]
PSUM tile (border taps read the zero padding):
  3x K=128: vertical tap pairs (0,b)+(1,b) from xb        (b = 0,1,2)
  1x K=128: horizontal tap pair (2,0)+(2,1) from xc at +2 rows
  1x K=64 : tap (2,2) from xb lower at +2 rows, col 2
Four blocks are processed pass-major so consecutive matmuls rotate
over 4 PSUM banks and pipeline through the PE array (same-bank
accumulation serializes at ~465ns/matmul; rotated it runs at ~220ns).
PSUM eviction (fused bias add) is split 3:1 DVE:Act per half; output
streams out per 16 rows alternating between the two HWDGE rings.
Image 0's DMAs and copies are chunked so the first matmul starts
early.
"""
import sys

for _p in ("/opt/trn_rl_repo", "/root/.axon_site/_ro/trn_rl_repo"):
    if _p not in sys.path:
        sys.path.append(_p)

import numpy as np
import ml_dtypes
from contextlib import ExitStack

import concourse.bacc as bacc
import concourse.tile as tile
from concourse import mybir
from concourse.bass_utils import run_bass_kernel_spmd

f32 = mybir.dt.float32
bf16 = mybir.dt.bfloat16

N_CORES = 8
NB = 4  # images per core


def build_nc():
    nc = bacc.Bacc()
    xp = nc.declare_dram_parameter("xp", [NB, 64, 66, 66], bf16, isOutput=False)
    wbh = nc.declare_dram_parameter("wbh", [128, 9, 128], bf16, isOutput=False)
    wch = nc.declare_dram_parameter("wch", [128, 128], bf16, isOutput=False)
    bias = nc.declare_dram_parameter("b", [128, 1], f32, isOutput=False)
    out = nc.declare_dram_parameter("out", [NB, 128, 64, 64], f32, isOutput=True)

    with tile.TileContext(nc) as tc, ExitStack() as ctx:
        const = ctx.enter_context(tc.tile_pool(name="const", bufs=1))
        xb_pool = ctx.enter_context(tc.tile_pool(name="xb", bufs=4))
        xc_pool = ctx.enter_context(tc.tile_pool(name="xc", bufs=2))
        ob_pool = ctx.enter_context(tc.tile_pool(name="ob", bufs=2))
        ps_pool = ctx.enter_context(tc.tile_pool(name="ps", bufs=2, space="PSUM"))

        # ---- weights, pre-arranged bf16 on the host.  wb [128, 9, 128]:
        # partition p<64 holds channel p's taps 0..8; partition 64+ci holds
        # taps 3..8 at slots 0..5, so wb[:, b, :] pairs taps (0,b) lower /
        # (1,b) upper and wb[0:64, 8, :] is tap (2,2).  wc [128, 128] pairs
        # taps (2,0) lower / (2,1) upper.
        wb = const.tile([128, 9, 128], bf16)
        wc = const.tile([128, 128], bf16)
        bt = const.tile([128, 1], f32)
        nc.scalar.dma_start(out=wb[:], in_=wbh[:])
        nc.scalar.dma_start(out=wc[:], in_=wch[:])
        nc.scalar.dma_start(out=bt[:], in_=bias[:])

        def chunks_for(n):
            # image 0 is on the critical path, so its loads are chunked
            # (half 0 of the matmuls reads padded rows <= 34)
            if n == 0:
                return ((0, 35), (35, 66)), ((0, 33), (33, 64))
            return ((0, 66),), ((0, 64),)

        def emit_image_dmas(n):
            """DMA xp[n] into a fresh xb tile (lower + row-shifted upper)."""
            xb = xb_pool.tile([128, 66, 66], bf16)
            chl, chu = chunks_for(n)
            for (t0, t1), (u0, u1) in zip(chl, chu):
                nc.sync.dma_start(out=xb[0:64, t0:t1, :], in_=xp[n][:, t0:t1, :])
                nc.sync.dma_start(
                    out=xb[64:128, u0:u1, :], in_=xp[n][:, u0 + 1:u1 + 1, :])
            return xb

        def emit_image_copies(n, xb):
            """Derive xc from xb: lower = xb lower (DVE); upper = one col
            left (img[r-1, j]), split between DVE and Act."""
            xc = xc_pool.tile([128, 66, 66], bf16)
            chl, _ = chunks_for(n)
            for t0, t1 in chl:
                tm = (t0 + t1) // 2
                nc.vector.tensor_copy(xc[0:64, t0:t1, :], xb[0:64, t0:t1, :])
                nc.vector.tensor_copy(xc[64:128, t0:tm, 0:65], xb[0:64, t0:tm, 1:66])
                nc.scalar.copy(xc[64:128, tm:t1, 0:65], xb[0:64, tm:t1, 1:66])
            return xc

        xb_cur = emit_image_dmas(0)
        tiles = (xb_cur, emit_image_copies(0, xb_cur))
        for n in range(NB):
            xb, xc = tiles
            if n + 1 < NB:
                # issue next image's input DMAs now: they get maximum lead
                # on the SP ring, ahead of this image's output pushes
                xb_next = emit_image_dmas(n + 1)

            osb = ob_pool.tile([128, 64, 64], f32)
            for half in range(2):
                # pass-major over 4 blocks: consecutive matmuls rotate over 4
                # PSUM banks, pipelining the PE and reusing each weight 4x
                P0 = ps_pool.tile([128, 8, 64], f32)
                P1 = ps_pool.tile([128, 8, 64], f32)
                P2 = ps_pool.tile([128, 8, 64], f32)
                P3 = ps_pool.tile([128, 8, 64], f32)
                Ps = (P0, P1, P2, P3)
                ys = [half * 32 + q * 8 for q in range(4)]
                for p in range(5):
                    st, sp = (p == 0), (p == 4)
                    for P, y0 in zip(Ps, ys):
                        if p < 3:
                            nc.tensor.matmul(
                                P[:, :, :], wb[:, p, :],
                                xb[:, y0:y0 + 8, p:p + 64],
                                start=st, stop=sp,
                            )
                        elif p == 3:
                            nc.tensor.matmul(
                                P[:, :, :], wc[:, :],
                                xc[:, y0 + 2:y0 + 10, 0:64],
                                start=st, stop=sp,
                            )
                        else:
                            nc.tensor.matmul(
                                P[:, :, :], wb[0:64, 8, :],
                                xb[0:64, y0 + 2:y0 + 10, 2:66],
                                start=st, stop=sp,
                            )
                for q, (P, y0) in enumerate(zip(Ps, ys)):
                    if q % 2 == 0:
                        nc.vector.tensor_scalar_add(osb[:, y0:y0 + 8, :], P[:, :, :], bt[:])
                    else:
                        nc.scalar.add(osb[:, y0:y0 + 8, :], P[:, :, :], bt[:])
                    if q % 2 == 1:
                        # outputs mostly ride the Act ring so inputs keep the
                        # SP ring clear; the final chunk of each image takes
                        # the (then idle) SP ring to shorten the tail
                        dma_eng = nc.sync if (half, q) == (1, 3) else nc.scalar
                        dma_eng.dma_start(
                            out=out[n][:, y0 - 8:y0 + 8, :],
                            in_=osb[:, y0 - 8:y0 + 8, :])
                if half == 0 and n + 1 < NB:
                    # next image's xc copies run on DVE/Act during this
                    # image's second half of matmuls
                    tiles = (xb_next, emit_image_copies(n + 1, xb_next))

    nc.finalize()
    return nc


_NC = None


def _get_nc():
    global _NC
    if _NC is None:
        _NC = build_nc()
    return _NC


def kernel(**inputs) -> np.ndarray:
    x = np.asarray(inputs["input"], dtype=np.float32)
    w = np.asarray(inputs["weight"], dtype=np.float32)
    b = np.ascontiguousarray(
        np.asarray(inputs["bias"], dtype=np.float32).reshape(128, 1))
    # host-side bf16 cast + zero padding: xp[n, c, t, j] = x[n, c, t-1, j-1]
    N = x.shape[0]
    xp = np.zeros((N, 64, 66, 66), dtype=ml_dtypes.bfloat16)
    xp[:, :, 1:65, 1:65] = x.astype(ml_dtypes.bfloat16)
    # weights in the kernel's partition layout (see build_nc)
    w3 = w.reshape(64, 9, 128).astype(ml_dtypes.bfloat16)
    wbh = np.zeros((128, 9, 128), dtype=ml_dtypes.bfloat16)
    wbh[0:64] = w3
    wbh[64:128, 0:6] = w3[:, 3:9]
    wch = np.concatenate([w3[:, 6], w3[:, 7]], axis=0)
    wch = np.ascontiguousarray(wch, dtype=ml_dtypes.bfloat16)
    nc = _get_nc()
    in_maps = [
        {"xp": xp[c * NB:(c + 1) * NB], "wbh": wbh, "wch": wch, "b": b}
        for c in range(N_CORES)
    ]
    res = run_bass_kernel_spmd(nc, in_maps, list(range(N_CORES)))
    return np.concatenate([r["out"] for r in res.results], axis=0)
